# revision 14
# baseline (speedup 1.0000x reference)
"""Trainium2 Bass kernel for nn_NeuralAttention (cross-attention with RoPE).

Sharding: 8 cores = 4 batches (data parallel) x 2 head-groups (tensor
parallel, 8 heads each).  Per-pair AllGather of the normalized per-head
attention outputs BEFORE the output projection; each core then computes
the full 16-head output projection for its half of the output columns
and writes that half directly (no AllReduce).

Per-core device program (SPMD, per-core data):
  phase A: Q projection + RoPE (starts ~5us; chunked weight/latent DMAs),
           V projection with pair-0 K'+RoPE interleaved.  All tables
           (cos/sin for keys) are host-gathered, no on-device gather.
  phase B: per head-pair: row-packed score matmuls (d=64 contraction,
           2x concurrent via PE row tiling), Exp on ScalarE with fused
           1/8 scale + per-key mask bias, attn@V matmuls with an appended
           ones column producing the softmax denominator for free;
           normalization via reciprocal_approx_fast + ones-broadcast
           matmul (off the score-PSUM pool).
  phase C: AllGather [my 8 heads] <-> pair core (512KB bf16), then
           output projection with K=128 head-pair-packed contraction,
           bias, DMA of the column half to DRAM.
"""

import numpy as np
import ml_dtypes

import concourse.bass as bass
import concourse.mybir as mybir
from concourse import bacc
import concourse.tile as tile
from concourse.bass_utils import run_bass_kernel_spmd

B, L, T = 4, 512, 4096
HID, NH, HD = 1024, 16, 64
MAX_POS, BASE = 4096, 10000.0
G = 2                 # TP head groups
NHG = NH // G         # heads per group
C = NHG * HD          # channels per group = 512
NCORES = 8
NSL = T // 512        # 8 key slices of 512
NT = T // 128         # 32 key tiles of 128

F32 = mybir.dt.float32
BF16 = mybir.dt.bfloat16

MULT = None
ADD = None

_BF = ml_dtypes.bfloat16


# ---------------------------------------------------------------- host prep
def _host_tables():
    inv_freq = 1.0 / BASE ** (np.arange(0, HD, 2, dtype=np.float32) / HD)
    t = np.arange(MAX_POS, dtype=np.float32)
    freqs = np.einsum('i,j->ij', t, inv_freq).astype(np.float32)
    emb = np.concatenate([freqs, freqs], axis=-1)          # [MAX_POS, HD]
    return np.cos(emb).astype(np.float32), np.sin(emb).astype(np.float32)


def _rot_perm2():
    # P: rotate_half as a linear map; P2 = blockdiag(P, P)  [128, 128]
    P = np.zeros((HD, HD), np.float32)
    for d in range(HD // 2):
        P[d, d + HD // 2] = -1.0
        P[d + HD // 2, d] = 1.0
    P2 = np.zeros((128, 128), np.float32)
    P2[:64, :64] = P
    P2[64:, 64:] = P
    return P2


# ---------------------------------------------------------------- bass build
_NC_CACHE = {}
OPTS = {"no_cc": False}


def _build_nc():
    global MULT, ADD
    MULT = mybir.AluOpType.mult
    ADD = mybir.AluOpType.add
    EXP = mybir.ActivationFunctionType.Exp

    nc = bacc.Bacc(None, target_bir_lowering=False)

    # -------- DRAM parameters (per-core data fed via in_maps)
    lat8 = nc.declare_dram_parameter("lat8", [8, 128, L], BF16, isOutput=False)
    wq8 = nc.declare_dram_parameter("wq8", [8, 128, C], BF16, isOutput=False)
    bqw = nc.declare_dram_parameter("bqw", [128, C // 128], F32, isOutput=False)
    cosq = nc.declare_dram_parameter("cosq", [128, L], BF16, isOutput=False)
    sinq = nc.declare_dram_parameter("sinq", [128, L], BF16, isOutput=False)
    pt2 = nc.declare_dram_parameter("pt2", [128, 128], BF16, isOutput=False)
    mbias = nc.declare_dram_parameter("mbias", [128, NT], F32, isOutput=False)

    wk1 = nc.declare_dram_parameter("wk1", [128, 8 * C], BF16, isOutput=False)
    bkw = nc.declare_dram_parameter("bkw", [128, C // 128], F32, isOutput=False)
    tgt8 = nc.declare_dram_parameter("tgt8", [NSL, 128, T], BF16, isOutput=False)
    wv1 = nc.declare_dram_parameter("wv1", [128, 8 * C], BF16, isOutput=False)
    bvrep = nc.declare_dram_parameter("bvrep", [128, C], F32, isOutput=False)
    cosk8 = nc.declare_dram_parameter("cosk8", [NSL, 128, 512], BF16, isOutput=False)
    sink8 = nc.declare_dram_parameter("sink8", [NSL, 128, 512], BF16, isOutput=False)

    wo2p = nc.declare_dram_parameter("wo2", [128, NHG * 512], BF16, isOutput=False)
    borep = nc.declare_dram_parameter("borep", [128, 512], F32, isOutput=False)

    out = nc.declare_dram_parameter("out", [L, 512], F32, isOutput=True)
    hx_in = nc.dram_tensor("hx_in", [64, NHG * L], BF16)
    hx_out = nc.dram_tensor("hx_out", [128, NHG * L], BF16)

    def mmr(out_ap, lhsT, rhs, **kw):
        nc.tensor.matmul(out_ap, lhsT, rhs, **kw)

    with tile.TileContext(nc) as tc:
        with tc.tile_pool(name="persist", bufs=1) as persist:
            # persistent tiles
            kpr = [persist.tile([128, T], BF16, tag=f"kpr{i}", name=f"kpr{i}")
                   for i in range(2)]
            qpr = [persist.tile([128, L], BF16, tag=f"qpr{i}", name=f"qpr{i}")
                   for i in range(4)]
            v_sb = persist.tile([128, NT, NHG, HD + 1], BF16, tag="v_sb")
            hT2 = persist.tile([128, NHG, L], BF16, tag="hT2")
            ones_sb = persist.tile([128, 64], F32, tag="ones")
            mb_sb = persist.tile([128, NT], F32, tag="mb")
            pt2_sb = persist.tile([128, 128], BF16, tag="pt2")
            wk_sb = persist.tile([128, 8, C], BF16, tag="wk")
            bk_sb = persist.tile([128, C // 128], F32, tag="bk")
            tg2 = persist.tile([128, NSL, 8, 512], BF16, tag="tg2")
            wv_sb = persist.tile([128, 8, C], BF16, tag="wv")
            bv_sb = persist.tile([128, C], F32, tag="bv")
            cosk_sb = persist.tile([128, T], BF16, tag="cosk")
            sink_sb = persist.tile([128, T], BF16, tag="sink")
            wo2_sb = persist.tile([128, NHG, 512], BF16, tag="wo2")
            bo_sb = persist.tile([128, 512], F32, tag="bo")

            scr_cm = tc.tile_pool(name="scr", bufs=3)
            scr = scr_cm.__enter__()
            qc_cm = tc.tile_pool(name="qc", bufs=1)
            qc = qc_cm.__enter__()
            lat_sb = qc.tile([128, 8, L], BF16, tag="lat")
            wq_sb = qc.tile([128, 8, C], BF16, tag="wq")
            bq_sb = qc.tile([128, C // 128], F32, tag="bq")
            cq_sb = qc.tile([128, L], BF16, tag="cq")
            sq_sb = qc.tile([128, L], BF16, tag="sq")

            # ---- all input DMAs issued up front, in priority order
            # (Q-projection dependencies first so the PE can start ~5us in)
            for k in range(8):
                nc.sync.dma_start(out=lat_sb[:, k, :], in_=lat8[k, :, :])
                nc.sync.dma_start(out=wq_sb[:, k, :], in_=wq8[k, :, :])
            nc.sync.dma_start(out=bq_sb, in_=bqw[:, :])
            nc.sync.dma_start(out=cq_sb, in_=cosq[:, :])
            nc.sync.dma_start(out=sq_sb, in_=sinq[:, :])
            nc.sync.dma_start(out=pt2_sb, in_=pt2[:, :])
            nc.sync.dma_start(out=mb_sb, in_=mbias[:, :])
            nc.sync.dma_start(
                out=wk_sb, in_=wk1[:, :].rearrange("p (k c) -> p k c", k=8))
            nc.sync.dma_start(out=bk_sb, in_=bkw[:, :])
            nc.sync.dma_start(
                out=tg2[:, 0, :, :],
                in_=tgt8[0, :, :].rearrange("p (k t) -> p k t", k=8))
            nc.sync.dma_start(
                out=wv_sb, in_=wv1[:, :].rearrange("p (k c) -> p k c", k=8))
            nc.sync.dma_start(out=bv_sb, in_=bvrep[:, :])
            for s in range(NSL):
                if s > 0:
                    nc.sync.dma_start(
                        out=tg2[:, s, :, :],
                        in_=tgt8[s, :, :].rearrange("p (k t) -> p k t", k=8))
                nc.sync.dma_start(
                    out=cosk_sb[:, s * 512:(s + 1) * 512], in_=cosk8[s, :, :])
                nc.sync.dma_start(
                    out=sink_sb[:, s * 512:(s + 1) * 512], in_=sink8[s, :, :])
            nc.sync.dma_start(
                out=wo2_sb, in_=wo2p[:, :].rearrange("p (h c) -> p h c", h=NHG))
            nc.sync.dma_start(out=bo_sb, in_=borep[:, :])

            nc.vector.memset(ones_sb[64:65, :], 1.0)
            nc.vector.memset(v_sb[:, :, :, HD:HD + 1], 1.0)

            # ---- Q projection + rope (first PE work)
            with tc.tile_pool(name="qps", bufs=2, space="PSUM") as qps, \
                 tc.tile_pool(name="qrp", bufs=2, space="PSUM") as qrp:
                for ct in range(4):
                    qp = qps.tile([128, L], F32, tag="qp")
                    for k in range(8):
                        mmr(qp, wq_sb[:, k, ct * 128:(ct + 1) * 128],
                            lat_sb[:, k, :], start=(k == 0), stop=(k == 7))
                    qsb = scr.tile([128, L], BF16, tag="ksb")
                    nc.vector.tensor_scalar_add(qsb, qp, bq_sb[:, ct:ct + 1])
                    qr = qrp.tile([128, L], F32, tag="qr")
                    mmr(qr, pt2_sb, qsb, start=True, stop=True)
                    t1 = scr.tile([128, L], BF16, tag="t1")
                    nc.vector.tensor_tensor(t1, qsb, cq_sb, MULT)
                    t2 = scr.tile([128, L], BF16, tag="t2")
                    nc.vector.tensor_tensor(t2, qr, sq_sb, MULT)
                    nc.vector.tensor_tensor(qpr[ct], t1, t2, ADD)
            qc_cm.__exit__(None, None, None)

            # ---- K' helpers (k' for pair pn, slice s of 512)
            kps_cm = tc.tile_pool(name="kps", bufs=1, space="PSUM")
            kps = kps_cm.__enter__()
            rps_cm = tc.tile_pool(name="rps", bufs=1, space="PSUM")
            rps = rps_cm.__enter__()

            def emit_kgroup(pn, s):
                kp = kps.tile([128, 512], F32, tag="kp", name="kp")
                for k in range(8):
                    mmr(kp, wk_sb[:, k, pn * 128:(pn + 1) * 128],
                        tg2[:, s, k, :], start=(k == 0), stop=(k == 7))
                ksb = scr.tile([128, 512], BF16, tag="ksb", name="ksb")
                nc.vector.tensor_scalar_add(ksb, kp, bk_sb[:, pn:pn + 1])
                return ksb

            def emit_krope(s, ksb, kdst):
                kr = rps.tile([128, 512], F32, tag="kr", name="kr")
                mmr(kr, pt2_sb, ksb, start=True, stop=True)
                t1 = scr.tile([128, 512], BF16, tag="t1", name="t1")
                nc.vector.tensor_tensor(t1, ksb, cosk_sb[:, s * 512:(s + 1) * 512], MULT)
                t2 = scr.tile([128, 512], BF16, tag="t2", name="t2")
                nc.vector.tensor_tensor(t2, kr, sink_sb[:, s * 512:(s + 1) * 512], MULT)
                nc.vector.tensor_tensor(kdst[:, s * 512:(s + 1) * 512], t1, t2, ADD)

            # ---- V projection with pair-0 K' interleaved
            with tc.tile_pool(name="vps", bufs=2, space="PSUM") as vps:
                ksb_pend = None
                for tt in range(NT):
                    if tt % 4 == 0:
                        ksb_pend = emit_kgroup(0, tt // 4)
                    vp = vps.tile([128, C], F32, tag="vp")
                    for k in range(8):
                        mmr(vp, tg2[:, tt // 4, k, (tt % 4) * 128:(tt % 4 + 1) * 128],
                            wv_sb[:, k, :], start=(k == 0), stop=(k == 7))
                    nc.vector.tensor_tensor(
                        v_sb[:, tt, :, 0:HD],
                        vp.rearrange("p (h d) -> p h d", h=NHG),
                        bv_sb.rearrange("p (h d) -> p h d", h=NHG), ADD)
                    if tt % 4 == 3:
                        emit_krope(tt // 4, ksb_pend, kpr[0])

            # ===== phase B: per-pair attention, next pair's K' interleaved
            sps_cm = tc.tile_pool(name="sps", bufs=3, space="PSUM")
            sps = sps_cm.__enter__()
            avp_cm = tc.tile_pool(name="avp", bufs=1, space="PSUM")
            avp = avp_cm.__enter__()
            bcp_cm = tc.tile_pool(name="bcp", bufs=1, space="PSUM")
            bcp = bcp_cm.__enter__()
            escr_cm = tc.tile_pool(name="escr", bufs=3)
            escr = escr_cm.__enter__()
            scr2_cm = tc.tile_pool(name="scr2", bufs=2)
            scr2 = scr2_cm.__enter__()
            for p in range(4):
                hA, hB = 2 * p, 2 * p + 1
                kcur = kpr[p % 2]
                knext = kpr[(p + 1) % 2]
                avA = avp.tile([65, L], F32, tag="avA", name="avA")
                avB = avp.tile([65, L], F32, tag="avB", name="avB")
                es = {}
                ksb_pend = None
                for tt in range(NT):
                    sA = sps.tile([128, L], F32, tag="s", name="sA")
                    nc.tensor.matmul(sA,
                                     kcur[0:64, tt * 128:(tt + 1) * 128],
                                     qpr[p][0:64, :], start=True, stop=True)
                    sB = sps.tile([128, L], F32, tag="s", name="sB")
                    nc.tensor.matmul(sB,
                                     kcur[64:128, tt * 128:(tt + 1) * 128],
                                     qpr[p][64:128, :], start=True, stop=True)
                    eAB = escr.tile([128, 2, L], BF16, tag="eAB", name="eAB")
                    nc.scalar.activation(out=eAB[:, 0, :], in_=sA, func=EXP,
                                         bias=mb_sb[:, tt:tt + 1], scale=0.125)
                    nc.scalar.activation(out=eAB[:, 1, :], in_=sB, func=EXP,
                                         bias=mb_sb[:, tt:tt + 1], scale=0.125)
                    es[tt] = eAB
                    if tt > 0:
                        eP = es.pop(tt - 1)
                        nc.tensor.matmul(avA, v_sb[:, tt - 1, hA, :], eP[:, 0, :],
                                         start=(tt - 1 == 0), stop=False)
                        nc.tensor.matmul(avB, v_sb[:, tt - 1, hB, :], eP[:, 1, :],
                                         start=(tt - 1 == 0), stop=False)
                    if p < 3:
                        if tt % 4 == 0:
                            ksb_pend = emit_kgroup(p + 1, tt // 4)
                        elif tt % 4 == 2:
                            emit_krope(tt // 4, ksb_pend, knext)
                eP = es.pop(NT - 1)
                nc.tensor.matmul(avA, v_sb[:, NT - 1, hA, :], eP[:, 0, :],
                                 start=False, stop=True)
                nc.tensor.matmul(avB, v_sb[:, NT - 1, hB, :], eP[:, 1, :],
                                 start=False, stop=True)

                # normalization: denominator is row 64 of av; broadcast its
                # reciprocal over the 64 output rows via a ones-column matmul
                # allocated from rps (NOT sps -- keeps next pair's scores
                # independent of this chain).
                for av, h in ((avA, hA), (avB, hB)):
                    dn = scr2.tile([128, L], F32, tag="dn", name="dn")
                    nc.vector.tensor_copy(out=dn[64:65, :], in_=av[64:65, :])
                    nc.vector.reciprocal(
                        out=dn[64:65, :], in_=dn[64:65, :])
                    osb = scr2.tile([64, L], BF16, tag="osb", name="osb")
                    nc.vector.tensor_copy(out=osb, in_=av[0:64, :])
                    bc = bcp.tile([64, 512], F32, tag="bc", name="bc")
                    nc.tensor.matmul(bc, ones_sb[64:65, :],
                                     dn[64:65, :], start=True, stop=True)
                    nc.vector.tensor_tensor(hT2[0:64, h, :], osb, bc, MULT)

            scr2_cm.__exit__(None, None, None)
            escr_cm.__exit__(None, None, None)
            bcp_cm.__exit__(None, None, None)
            avp_cm.__exit__(None, None, None)
            sps_cm.__exit__(None, None, None)
            rps_cm.__exit__(None, None, None)
            kps_cm.__exit__(None, None, None)
            scr_cm.__exit__(None, None, None)

            # ===== phase C: exchange heads with pair core, output projection
            nc.sync.dma_start(
                out=hx_in[:, :],
                in_=hT2[0:64, :, :].rearrange("p h l -> p (h l)"))
            if OPTS["no_cc"]:
                nc.sync.dma_start(
                    out=hT2[:, :, :],
                    in_=hx_in[:, :].rearrange("p (h l) -> p h l", h=NHG))
                nc.sync.dma_start(
                    out=hT2[64:128, :, :],
                    in_=hx_in[:, :].rearrange("p (h l) -> p h l", h=NHG))
            else:
                nc.gpsimd.collective_compute(
                    "AllGather", mybir.AluOpType.bypass,
                    ins=[hx_in[:, :]], outs=[hx_out[:, :]],
                    replica_groups=[[0, 1], [2, 3], [4, 5], [6, 7]],
                )
                # rows 0:64 = g0 chunk, 64:128 = g1 chunk (fixed layout on
                # both cores; own data round-trips through DRAM)
                nc.sync.dma_start(
                    out=hT2[:, :, :],
                    in_=hx_out[:, :].rearrange("p (h l) -> p h l", h=NHG))

            with tc.tile_pool(name="ops", bufs=1, space="PSUM") as ops, \
                 tc.tile_pool(name="ow", bufs=4) as ow:
                opst = [ops.tile([128, 512], F32, tag=f"op{lt}", name=f"op{lt}")
                        for lt in range(4)]
                for h in range(NHG):
                    for lt in range(4):
                        mmr(opst[lt], hT2[:, h, lt * 128:(lt + 1) * 128],
                            wo2_sb[:, h, :], start=(h == 0), stop=(h == NHG - 1))
                for lt in range(4):
                    ob = ow.tile([128, 512], F32, tag="ob", name="ob")
                    nc.vector.tensor_tensor(ob, opst[lt], bo_sb, ADD)
                    nc.sync.dma_start(
                        out=out[lt * 128:(lt + 1) * 128, :], in_=ob)

    return nc


def get_nc():
    key = tuple(sorted(OPTS.items()))
    if key not in _NC_CACHE:
        nc = _build_nc()
        if not nc.is_finalized():
            nc.finalize()
        _NC_CACHE[key] = nc
    return _NC_CACHE[key]


# ---------------------------------------------------------------- host side
def make_in_maps(latents, target, target_mask, target_timestamp,
                 Wq, bq, Wk, bk, Wv, bv, Wo, bo):
    cos_tab, sin_tab = _host_tables()
    P2 = _rot_perm2()

    lat_ts = (np.arange(L, dtype=np.float32) * (MAX_POS - 1) / (L - 1)).astype(np.int64)
    cosq = np.tile(cos_tab[lat_ts].T, (2, 1)).astype(_BF)   # [128, L]
    sinq = np.tile(sin_tab[lat_ts].T, (2, 1)).astype(_BF)
    pt2 = np.ascontiguousarray(P2.T).astype(_BF)

    WoT = np.ascontiguousarray(np.asarray(Wo).T)            # [1024, 1024]

    # per-batch shared prep
    tgt8_b, cosk8_b, sink8_b, mb_b = [], [], [], []
    for b in range(B):
        tgtT = np.asarray(target[b]).T                      # [1024, T]
        tgt8_b.append(np.ascontiguousarray(
            tgtT.reshape(8, 128, NSL, 512).transpose(2, 1, 0, 3)
                .reshape(NSL, 128, T)).astype(_BF))
        ts = np.asarray(target_timestamp[b]).astype(np.int64)
        ck = np.tile(cos_tab[ts].T, (2, 1))                  # [128, T]
        sk = np.tile(sin_tab[ts].T, (2, 1))
        cosk8_b.append(np.ascontiguousarray(
            ck.reshape(128, NSL, 512).transpose(1, 0, 2)).astype(_BF))
        sink8_b.append(np.ascontiguousarray(
            sk.reshape(128, NSL, 512).transpose(1, 0, 2)).astype(_BF))
        mask = np.asarray(target_mask[b]).astype(np.float32)
        mb_b.append(np.ascontiguousarray(
            ((mask - 1.0) * 30000.0).reshape(NT, 128).T).astype(np.float32))

    latT = np.asarray(latents).transpose(0, 2, 1)           # [B, 1024, L]

    in_maps = []
    for core in range(NCORES):
        b, g = core // 2, core % 2
        sl = slice(g * C, (g + 1) * C)
        csl = slice(g * 512, (g + 1) * 512)                  # output col half
        wqT = np.asarray(Wq)[sl, :].T                        # [1024, C]
        wkT = np.asarray(Wk)[sl, :].T
        wvT = np.asarray(Wv)[sl, :].T
        # wo2: rows [g0 h d | g1 h d] fixed order, cols = this core's half
        wo2 = np.ascontiguousarray(
            WoT.reshape(2, NHG, 64, HID)[:, :, :, csl]
               .transpose(0, 2, 1, 3).reshape(128, NHG * 512)).astype(_BF)
        m = {
            "lat8": np.ascontiguousarray(
                latT[b].reshape(8, 128, L)).astype(_BF),
            "wq8": np.ascontiguousarray(wqT.reshape(8, 128, C)).astype(_BF),
            "bqw": np.ascontiguousarray(
                np.asarray(bq)[sl].reshape(C // 128, 128).T.astype(np.float32)),
            "cosq": cosq, "sinq": sinq, "pt2": pt2,
            "mbias": mb_b[b],
            "wk1": np.ascontiguousarray(
                wkT.reshape(8, 128, C).transpose(1, 0, 2)
                   .reshape(128, 8 * C)).astype(_BF),
            "bkw": np.ascontiguousarray(
                np.asarray(bk)[sl].reshape(C // 128, 128).T.astype(np.float32)),
            "tgt8": tgt8_b[b],
            "wv1": np.ascontiguousarray(
                wvT.reshape(8, 128, C).transpose(1, 0, 2)
                   .reshape(128, 8 * C)).astype(_BF),
            "bvrep": np.ascontiguousarray(
                np.tile(np.asarray(bv)[sl][None, :], (128, 1)).astype(np.float32)),
            "cosk8": cosk8_b[b], "sink8": sink8_b[b],
            "wo2": wo2,
            "borep": np.ascontiguousarray(
                np.tile(np.asarray(bo)[csl][None, :], (128, 1)).astype(np.float32)),
        }
        in_maps.append(m)
    return in_maps


def kernel(latents, target, target_mask, target_timestamp,
           Wq, bq, Wk, bk, Wv, bv, Wo, bo, _trace=False, _trace_kwargs=None):
    in_maps = make_in_maps(latents, target, target_mask, target_timestamp,
                           Wq, bq, Wk, bk, Wv, bv, Wo, bo)
    nc = get_nc()
    res = run_bass_kernel_spmd(nc, in_maps, list(range(NCORES)),
                               trace=_trace, **(_trace_kwargs or {}))
    full = np.zeros((B, L, HID), np.float32)
    for b in range(B):
        full[b][:, 0:512] = res.results[2 * b]["out"]
        full[b][:, 512:1024] = res.results[2 * b + 1]["out"]
    if _trace:
        return full, res
    return full


# revision 17
# speedup vs baseline: 1.0835x; 1.0835x over previous
"""Trainium2 Bass kernel for nn_NeuralAttention (cross-attention with RoPE).

Sharding: 8 cores = 4 batches (data parallel) x 2 head-groups (tensor
parallel, 8 heads each).  Per-pair AllGather of the normalized per-head
attention outputs BEFORE the output projection; each core then computes
the full 16-head output projection for its half of the output columns
and writes that half directly (no AllReduce).

Per-core device program (SPMD, per-core data):
  phase A: Q projection + RoPE (starts ~5us; chunked weight/latent DMAs),
           V projection with pair-0 K'+RoPE interleaved.  All tables
           (cos/sin for keys) are host-gathered, no on-device gather.
  phase B: per head-pair: row-packed score matmuls (d=64 contraction,
           2x concurrent via PE row tiling), Exp on ScalarE with fused
           1/8 scale + per-key mask bias, attn@V matmuls with an appended
           ones column producing the softmax denominator for free;
           normalization via reciprocal_approx_fast + ones-broadcast
           matmul (off the score-PSUM pool).
  phase C: AllGather [my 8 heads] <-> pair core (512KB bf16), then
           output projection with K=128 head-pair-packed contraction,
           bias, DMA of the column half to DRAM.
"""

import numpy as np
import ml_dtypes

import concourse.bass as bass
import concourse.mybir as mybir
from concourse import bacc
import concourse.tile as tile
from concourse.bass_utils import run_bass_kernel_spmd

B, L, T = 4, 512, 4096
HID, NH, HD = 1024, 16, 64
MAX_POS, BASE = 4096, 10000.0
G = 2                 # TP head groups
NHG = NH // G         # heads per group
C = NHG * HD          # channels per group = 512
NCORES = 8
NSL = T // 512        # 8 key slices of 512
NT = T // 128         # 32 key tiles of 128

F32 = mybir.dt.float32
BF16 = mybir.dt.bfloat16

MULT = None
ADD = None

_BF = ml_dtypes.bfloat16


# ---------------------------------------------------------------- host prep
def _host_tables():
    inv_freq = 1.0 / BASE ** (np.arange(0, HD, 2, dtype=np.float32) / HD)
    t = np.arange(MAX_POS, dtype=np.float32)
    freqs = np.einsum('i,j->ij', t, inv_freq).astype(np.float32)
    emb = np.concatenate([freqs, freqs], axis=-1)          # [MAX_POS, HD]
    return np.cos(emb).astype(np.float32), np.sin(emb).astype(np.float32)


def _rot_perm2():
    # P: rotate_half as a linear map; P2 = blockdiag(P, P)  [128, 128]
    P = np.zeros((HD, HD), np.float32)
    for d in range(HD // 2):
        P[d, d + HD // 2] = -1.0
        P[d + HD // 2, d] = 1.0
    P2 = np.zeros((128, 128), np.float32)
    P2[:64, :64] = P
    P2[64:, 64:] = P
    return P2


# ---------------------------------------------------------------- bass build
_NC_CACHE = {}
OPTS = {"no_cc": False}


def _build_nc():
    global MULT, ADD
    MULT = mybir.AluOpType.mult
    ADD = mybir.AluOpType.add
    EXP = mybir.ActivationFunctionType.Exp

    nc = bacc.Bacc(None, target_bir_lowering=False)

    # -------- DRAM parameters (per-core data fed via in_maps)
    lat8 = nc.declare_dram_parameter("lat8", [8, 128, L], BF16, isOutput=False)
    wq8 = nc.declare_dram_parameter("wq8", [8, 128, C], BF16, isOutput=False)
    bqw = nc.declare_dram_parameter("bqw", [128, C // 128], F32, isOutput=False)
    cosq = nc.declare_dram_parameter("cosq", [128, L], BF16, isOutput=False)
    sinq = nc.declare_dram_parameter("sinq", [128, L], BF16, isOutput=False)
    pt2 = nc.declare_dram_parameter("pt2", [128, 128], BF16, isOutput=False)
    mbias = nc.declare_dram_parameter("mbias", [128, NT], F32, isOutput=False)

    wk1 = nc.declare_dram_parameter("wk1", [128, 8 * C], BF16, isOutput=False)
    bkw = nc.declare_dram_parameter("bkw", [128, C // 128], F32, isOutput=False)
    tgt8 = nc.declare_dram_parameter("tgt8", [NSL, 128, T], BF16, isOutput=False)
    wv1 = nc.declare_dram_parameter("wv1", [128, 8 * C], BF16, isOutput=False)
    bvrep = nc.declare_dram_parameter("bvrep", [128, C], F32, isOutput=False)
    cosk8 = nc.declare_dram_parameter("cosk8", [NSL, 128, 512], BF16, isOutput=False)
    sink8 = nc.declare_dram_parameter("sink8", [NSL, 128, 512], BF16, isOutput=False)

    wo2p = nc.declare_dram_parameter("wo2", [128, NHG * 512], BF16, isOutput=False)
    borep = nc.declare_dram_parameter("borep", [128, 512], F32, isOutput=False)

    out = nc.declare_dram_parameter("out", [L, 512], F32, isOutput=True)
    hxi = [nc.dram_tensor(f"hxi{p}", [64, 2 * L], BF16) for p in range(4)]
    hxo = [nc.dram_tensor(f"hxo{p}", [128, 2 * L], BF16) for p in range(4)]

    def mmr(out_ap, lhsT, rhs, **kw):
        nc.tensor.matmul(out_ap, lhsT, rhs, **kw)

    with tile.TileContext(nc) as tc:
        with tc.tile_pool(name="persist", bufs=1) as persist:
            # persistent tiles
            kpr = [persist.tile([128, T], BF16, tag=f"kpr{i}", name=f"kpr{i}")
                   for i in range(2)]
            qpr = [persist.tile([128, L], BF16, tag=f"qpr{i}", name=f"qpr{i}")
                   for i in range(4)]
            v_sb = persist.tile([128, NT, NHG, HD + 1], BF16, tag="v_sb")
            hT2 = persist.tile([128, NHG, L], BF16, tag="hT2")
            ones_sb = persist.tile([128, 64], F32, tag="ones")
            mb_sb = persist.tile([128, NT], F32, tag="mb")
            pt2_sb = persist.tile([128, 128], BF16, tag="pt2")
            wk_sb = persist.tile([128, 8, C], BF16, tag="wk")
            bk_sb = persist.tile([128, C // 128], F32, tag="bk")
            tg2 = persist.tile([128, NSL, 8, 512], BF16, tag="tg2")
            wv_sb = persist.tile([128, 8, C], BF16, tag="wv")
            bv_sb = persist.tile([128, C], F32, tag="bv")
            cosk_sb = persist.tile([128, T], BF16, tag="cosk")
            sink_sb = persist.tile([128, T], BF16, tag="sink")
            wo2_sb = persist.tile([128, NHG, 512], BF16, tag="wo2")
            bo_sb = persist.tile([128, 512], F32, tag="bo")

            scr_cm = tc.tile_pool(name="scr", bufs=3)
            scr = scr_cm.__enter__()
            qc_cm = tc.tile_pool(name="qc", bufs=1)
            qc = qc_cm.__enter__()
            lat_sb = qc.tile([128, 8, L], BF16, tag="lat")
            wq_sb = qc.tile([128, 8, C], BF16, tag="wq")
            bq_sb = qc.tile([128, C // 128], F32, tag="bq")
            cq_sb = qc.tile([128, L], BF16, tag="cq")
            sq_sb = qc.tile([128, L], BF16, tag="sq")

            # ---- all input DMAs issued up front, in priority order
            # (Q-projection dependencies first so the PE can start ~5us in)
            for k in range(8):
                nc.sync.dma_start(out=lat_sb[:, k, :], in_=lat8[k, :, :])
                nc.sync.dma_start(out=wq_sb[:, k, :], in_=wq8[k, :, :])
            nc.sync.dma_start(out=bq_sb, in_=bqw[:, :])
            nc.sync.dma_start(out=cq_sb, in_=cosq[:, :])
            nc.sync.dma_start(out=sq_sb, in_=sinq[:, :])
            nc.sync.dma_start(out=pt2_sb, in_=pt2[:, :])
            nc.sync.dma_start(out=mb_sb, in_=mbias[:, :])
            nc.sync.dma_start(
                out=wk_sb, in_=wk1[:, :].rearrange("p (k c) -> p k c", k=8))
            nc.sync.dma_start(out=bk_sb, in_=bkw[:, :])
            nc.sync.dma_start(
                out=tg2[:, 0, :, :],
                in_=tgt8[0, :, :].rearrange("p (k t) -> p k t", k=8))
            nc.sync.dma_start(
                out=wv_sb, in_=wv1[:, :].rearrange("p (k c) -> p k c", k=8))
            nc.sync.dma_start(out=bv_sb, in_=bvrep[:, :])
            for s in range(NSL):
                if s > 0:
                    nc.sync.dma_start(
                        out=tg2[:, s, :, :],
                        in_=tgt8[s, :, :].rearrange("p (k t) -> p k t", k=8))
                nc.sync.dma_start(
                    out=cosk_sb[:, s * 512:(s + 1) * 512], in_=cosk8[s, :, :])
                nc.sync.dma_start(
                    out=sink_sb[:, s * 512:(s + 1) * 512], in_=sink8[s, :, :])
            nc.sync.dma_start(
                out=wo2_sb, in_=wo2p[:, :].rearrange("p (h c) -> p h c", h=NHG))
            nc.sync.dma_start(out=bo_sb, in_=borep[:, :])

            nc.vector.memset(ones_sb[64:65, :], 1.0)
            nc.vector.memset(v_sb[:, :, :, HD:HD + 1], 1.0)

            # ---- Q projection + rope (first PE work)
            with tc.tile_pool(name="qps", bufs=2, space="PSUM") as qps, \
                 tc.tile_pool(name="qrp", bufs=2, space="PSUM") as qrp:
                for ct in range(4):
                    qp = qps.tile([128, L], F32, tag="qp")
                    for k in range(8):
                        mmr(qp, wq_sb[:, k, ct * 128:(ct + 1) * 128],
                            lat_sb[:, k, :], start=(k == 0), stop=(k == 7))
                    qsb = scr.tile([128, L], BF16, tag="ksb")
                    nc.vector.tensor_scalar_add(qsb, qp, bq_sb[:, ct:ct + 1])
                    qr = qrp.tile([128, L], F32, tag="qr")
                    mmr(qr, pt2_sb, qsb, start=True, stop=True)
                    t1 = scr.tile([128, L], BF16, tag="t1")
                    nc.vector.tensor_tensor(t1, qsb, cq_sb, MULT)
                    t2 = scr.tile([128, L], BF16, tag="t2")
                    nc.vector.tensor_tensor(t2, qr, sq_sb, MULT)
                    nc.vector.tensor_tensor(qpr[ct], t1, t2, ADD)
            qc_cm.__exit__(None, None, None)

            # ---- K' helpers (k' for pair pn, slice s of 512)
            kps_cm = tc.tile_pool(name="kps", bufs=1, space="PSUM")
            kps = kps_cm.__enter__()
            rps_cm = tc.tile_pool(name="rps", bufs=1, space="PSUM")
            rps = rps_cm.__enter__()

            def emit_kgroup(pn, s):
                kp = kps.tile([128, 512], F32, tag="kp", name="kp")
                for k in range(8):
                    mmr(kp, wk_sb[:, k, pn * 128:(pn + 1) * 128],
                        tg2[:, s, k, :], start=(k == 0), stop=(k == 7))
                ksb = scr.tile([128, 512], BF16, tag="ksb", name="ksb")
                nc.vector.tensor_scalar_add(ksb, kp, bk_sb[:, pn:pn + 1])
                return ksb

            def emit_krope(s, ksb, kdst):
                kr = rps.tile([128, 512], F32, tag="kr", name="kr")
                mmr(kr, pt2_sb, ksb, start=True, stop=True)
                t1 = scr.tile([128, 512], BF16, tag="t1", name="t1")
                nc.vector.tensor_tensor(t1, ksb, cosk_sb[:, s * 512:(s + 1) * 512], MULT)
                t2 = scr.tile([128, 512], BF16, tag="t2", name="t2")
                nc.vector.tensor_tensor(t2, kr, sink_sb[:, s * 512:(s + 1) * 512], MULT)
                nc.vector.tensor_tensor(kdst[:, s * 512:(s + 1) * 512], t1, t2, ADD)

            # ---- V projection with pair-0 K' interleaved
            with tc.tile_pool(name="vps", bufs=2, space="PSUM") as vps:
                ksb_pend = None
                for tt in range(NT):
                    if tt % 4 == 0:
                        ksb_pend = emit_kgroup(0, tt // 4)
                    vp = vps.tile([128, C], F32, tag="vp")
                    for k in range(8):
                        mmr(vp, tg2[:, tt // 4, k, (tt % 4) * 128:(tt % 4 + 1) * 128],
                            wv_sb[:, k, :], start=(k == 0), stop=(k == 7))
                    nc.vector.tensor_tensor(
                        v_sb[:, tt, :, 0:HD],
                        vp.rearrange("p (h d) -> p h d", h=NHG),
                        bv_sb.rearrange("p (h d) -> p h d", h=NHG), ADD)
                    if tt % 4 == 3:
                        emit_krope(tt // 4, ksb_pend, kpr[0])

            # ===== phase B: per-pair attention, next pair's K' interleaved
            sps_cm = tc.tile_pool(name="sps", bufs=3, space="PSUM")
            sps = sps_cm.__enter__()
            avp_cm = tc.tile_pool(name="avp", bufs=1, space="PSUM")
            avp = avp_cm.__enter__()
            bcp_cm = tc.tile_pool(name="bcp", bufs=1, space="PSUM")
            bcp = bcp_cm.__enter__()
            escr_cm = tc.tile_pool(name="escr", bufs=3)
            escr = escr_cm.__enter__()
            scr2_cm = tc.tile_pool(name="scr2", bufs=2)
            scr2 = scr2_cm.__enter__()
            for p in range(4):
                hA, hB = 2 * p, 2 * p + 1
                kcur = kpr[p % 2]
                knext = kpr[(p + 1) % 2]
                avA = avp.tile([65, L], F32, tag="avA", name="avA")
                avB = avp.tile([65, L], F32, tag="avB", name="avB")
                es = {}
                ksb_pend = None
                for tt in range(NT):
                    sA = sps.tile([128, L], F32, tag="s", name="sA")
                    nc.tensor.matmul(sA,
                                     kcur[0:64, tt * 128:(tt + 1) * 128],
                                     qpr[p][0:64, :], start=True, stop=True)
                    sB = sps.tile([128, L], F32, tag="s", name="sB")
                    nc.tensor.matmul(sB,
                                     kcur[64:128, tt * 128:(tt + 1) * 128],
                                     qpr[p][64:128, :], start=True, stop=True)
                    eAB = escr.tile([128, 2, L], BF16, tag="eAB", name="eAB")
                    nc.scalar.activation(out=eAB[:, 0, :], in_=sA, func=EXP,
                                         bias=mb_sb[:, tt:tt + 1], scale=0.125)
                    nc.scalar.activation(out=eAB[:, 1, :], in_=sB, func=EXP,
                                         bias=mb_sb[:, tt:tt + 1], scale=0.125)
                    es[tt] = eAB
                    if tt > 0:
                        eP = es.pop(tt - 1)
                        nc.tensor.matmul(avA, v_sb[:, tt - 1, hA, :], eP[:, 0, :],
                                         start=(tt - 1 == 0), stop=False)
                        nc.tensor.matmul(avB, v_sb[:, tt - 1, hB, :], eP[:, 1, :],
                                         start=(tt - 1 == 0), stop=False)
                    if p < 3:
                        if tt % 4 == 0:
                            ksb_pend = emit_kgroup(p + 1, tt // 4)
                        elif tt % 4 == 2:
                            emit_krope(tt // 4, ksb_pend, knext)
                eP = es.pop(NT - 1)
                nc.tensor.matmul(avA, v_sb[:, NT - 1, hA, :], eP[:, 0, :],
                                 start=False, stop=True)
                nc.tensor.matmul(avB, v_sb[:, NT - 1, hB, :], eP[:, 1, :],
                                 start=False, stop=True)

                # normalization: denominator is row 64 of av.  Its reciprocal
                # is computed as exp(-ln(d)) on the otherwise-idle Scalar
                # engine (keeps the in-order DVE queue free so the copies
                # that release the attention PSUM run immediately), then
                # broadcast over the 64 output rows via a ones-column matmul.
                parts = []
                for av, h in ((avA, hA), (avB, hB)):
                    lnd = scr2.tile([128, L], F32, tag="lnd", name="lnd")
                    nc.scalar.activation(out=lnd[64:65, :], in_=av[64:65, :],
                                         func=mybir.ActivationFunctionType.Ln)
                    osb = scr2.tile([64, L], BF16, tag="osb", name="osb")
                    nc.vector.tensor_copy(out=osb, in_=av[0:64, :])
                    parts.append((lnd, osb, h))
                for lnd, osb, h in parts:
                    dnr = scr2.tile([128, L], F32, tag="dnr", name="dnr")
                    nc.scalar.activation(out=dnr[64:65, :], in_=lnd[64:65, :],
                                         func=EXP, scale=-1.0)
                    bc = bcp.tile([64, 512], F32, tag="bc", name="bc")
                    nc.tensor.matmul(bc, ones_sb[64:65, :],
                                     dnr[64:65, :], start=True, stop=True)
                    nc.vector.tensor_tensor(hT2[0:64, h, :], osb, bc, MULT)

                # exchange this pair's normalized heads with the pair core
                # (overlaps with the next pair's attention compute)
                nc.sync.dma_start(
                    out=hxi[p][:, :],
                    in_=hT2[0:64, 2 * p:2 * p + 2, :].rearrange("p h l -> p (h l)"))
                if OPTS["no_cc"]:
                    nc.sync.dma_start(
                        out=hT2[:, 2 * p:2 * p + 2, :],
                        in_=hxo[p][:, :].rearrange("p (h l) -> p h l", h=2))
                else:
                    nc.gpsimd.collective_compute(
                        "AllGather", mybir.AluOpType.bypass,
                        ins=[hxi[p][:, :]], outs=[hxo[p][:, :]],
                        replica_groups=[[0, 1], [2, 3], [4, 5], [6, 7]],
                    )
                    # rows 0:64 = g0 chunk, 64:128 = g1 chunk (fixed layout
                    # on both cores; own data round-trips through DRAM)
                    nc.sync.dma_start(
                        out=hT2[:, 2 * p:2 * p + 2, :],
                        in_=hxo[p][:, :].rearrange("p (h l) -> p h l", h=2))

            scr2_cm.__exit__(None, None, None)
            escr_cm.__exit__(None, None, None)
            bcp_cm.__exit__(None, None, None)
            avp_cm.__exit__(None, None, None)
            sps_cm.__exit__(None, None, None)
            rps_cm.__exit__(None, None, None)
            kps_cm.__exit__(None, None, None)
            scr_cm.__exit__(None, None, None)

            # ===== phase C: output projection (exchanges already in flight;
            # heads 0-5 typically land before phase B ends, heads 6-7 gate
            # only the last two accumulation steps)
            with tc.tile_pool(name="ops", bufs=1, space="PSUM") as ops, \
                 tc.tile_pool(name="ow", bufs=4) as ow:
                opst = [ops.tile([128, 512], F32, tag=f"op{lt}", name=f"op{lt}")
                        for lt in range(4)]
                for h in range(NHG):
                    for lt in range(4):
                        mmr(opst[lt], hT2[:, h, lt * 128:(lt + 1) * 128],
                            wo2_sb[:, h, :], start=(h == 0), stop=(h == NHG - 1))
                for lt in range(4):
                    ob = ow.tile([128, 512], F32, tag="ob", name="ob")
                    nc.vector.tensor_tensor(ob, opst[lt], bo_sb, ADD)
                    nc.sync.dma_start(
                        out=out[lt * 128:(lt + 1) * 128, :], in_=ob)

    return nc


def get_nc():
    key = tuple(sorted(OPTS.items()))
    if key not in _NC_CACHE:
        nc = _build_nc()
        if not nc.is_finalized():
            nc.finalize()
        _NC_CACHE[key] = nc
    return _NC_CACHE[key]


# ---------------------------------------------------------------- host side
def make_in_maps(latents, target, target_mask, target_timestamp,
                 Wq, bq, Wk, bk, Wv, bv, Wo, bo):
    cos_tab, sin_tab = _host_tables()
    P2 = _rot_perm2()

    lat_ts = (np.arange(L, dtype=np.float32) * (MAX_POS - 1) / (L - 1)).astype(np.int64)
    cosq = np.tile(cos_tab[lat_ts].T, (2, 1)).astype(_BF)   # [128, L]
    sinq = np.tile(sin_tab[lat_ts].T, (2, 1)).astype(_BF)
    pt2 = np.ascontiguousarray(P2.T).astype(_BF)

    WoT = np.ascontiguousarray(np.asarray(Wo).T)            # [1024, 1024]

    # per-batch shared prep
    tgt8_b, cosk8_b, sink8_b, mb_b = [], [], [], []
    for b in range(B):
        tgtT = np.asarray(target[b]).T                      # [1024, T]
        tgt8_b.append(np.ascontiguousarray(
            tgtT.reshape(8, 128, NSL, 512).transpose(2, 1, 0, 3)
                .reshape(NSL, 128, T)).astype(_BF))
        ts = np.asarray(target_timestamp[b]).astype(np.int64)
        ck = np.tile(cos_tab[ts].T, (2, 1))                  # [128, T]
        sk = np.tile(sin_tab[ts].T, (2, 1))
        cosk8_b.append(np.ascontiguousarray(
            ck.reshape(128, NSL, 512).transpose(1, 0, 2)).astype(_BF))
        sink8_b.append(np.ascontiguousarray(
            sk.reshape(128, NSL, 512).transpose(1, 0, 2)).astype(_BF))
        mask = np.asarray(target_mask[b]).astype(np.float32)
        mb_b.append(np.ascontiguousarray(
            ((mask - 1.0) * 30000.0).reshape(NT, 128).T).astype(np.float32))

    latT = np.asarray(latents).transpose(0, 2, 1)           # [B, 1024, L]

    in_maps = []
    for core in range(NCORES):
        b, g = core // 2, core % 2
        sl = slice(g * C, (g + 1) * C)
        csl = slice(g * 512, (g + 1) * 512)                  # output col half
        wqT = np.asarray(Wq)[sl, :].T                        # [1024, C]
        wkT = np.asarray(Wk)[sl, :].T
        wvT = np.asarray(Wv)[sl, :].T
        # wo2: rows [g0 h d | g1 h d] fixed order, cols = this core's half
        wo2 = np.ascontiguousarray(
            WoT.reshape(2, NHG, 64, HID)[:, :, :, csl]
               .transpose(0, 2, 1, 3).reshape(128, NHG * 512)).astype(_BF)
        m = {
            "lat8": np.ascontiguousarray(
                latT[b].reshape(8, 128, L)).astype(_BF),
            "wq8": np.ascontiguousarray(wqT.reshape(8, 128, C)).astype(_BF),
            "bqw": np.ascontiguousarray(
                np.asarray(bq)[sl].reshape(C // 128, 128).T.astype(np.float32)),
            "cosq": cosq, "sinq": sinq, "pt2": pt2,
            "mbias": mb_b[b],
            "wk1": np.ascontiguousarray(
                wkT.reshape(8, 128, C).transpose(1, 0, 2)
                   .reshape(128, 8 * C)).astype(_BF),
            "bkw": np.ascontiguousarray(
                np.asarray(bk)[sl].reshape(C // 128, 128).T.astype(np.float32)),
            "tgt8": tgt8_b[b],
            "wv1": np.ascontiguousarray(
                wvT.reshape(8, 128, C).transpose(1, 0, 2)
                   .reshape(128, 8 * C)).astype(_BF),
            "bvrep": np.ascontiguousarray(
                np.tile(np.asarray(bv)[sl][None, :], (128, 1)).astype(np.float32)),
            "cosk8": cosk8_b[b], "sink8": sink8_b[b],
            "wo2": wo2,
            "borep": np.ascontiguousarray(
                np.tile(np.asarray(bo)[csl][None, :], (128, 1)).astype(np.float32)),
        }
        in_maps.append(m)
    return in_maps


def kernel(latents, target, target_mask, target_timestamp,
           Wq, bq, Wk, bk, Wv, bv, Wo, bo, _trace=False, _trace_kwargs=None):
    in_maps = make_in_maps(latents, target, target_mask, target_timestamp,
                           Wq, bq, Wk, bk, Wv, bv, Wo, bo)
    nc = get_nc()
    res = run_bass_kernel_spmd(nc, in_maps, list(range(NCORES)),
                               trace=_trace, **(_trace_kwargs or {}))
    full = np.zeros((B, L, HID), np.float32)
    for b in range(B):
        full[b][:, 0:512] = res.results[2 * b]["out"]
        full[b][:, 512:1024] = res.results[2 * b + 1]["out"]
    if _trace:
        return full, res
    return full
